# revision 19
# baseline (speedup 1.0000x reference)
"""Causal multi-head attention on 8 TRN2 NeuronCores.

Reference (per batch b):
    q,k,v = x @ W^T  (W: [d_out, d_in]), split into H=16 heads of dk=64
    attn  = softmax(causal(q k^T / sqrt(dk))) v
    y     = concat_heads(attn) @ W_o^T

Sharding (8 cores): core c -> batch b = c//4, head group g = c%4 (4 heads,
256 channels).  w_q/w_k/w_v column-sharded by head; w_o row-sharded — each
core computes a partial y[b] over its 256 channels; the host sums the 4
partials per batch (the unshard step).

Device kernel structure (one software pipeline over s-chunks of 512):
    qk_proj(0) + v_proj(0) run up front; afterwards every remaining
    projection / y-projection is chopped into small PE "filler units"
    (qk_proj(j+1) x4, v_proj(j) x4, stage_d(j-1) x8) that are woven between
    the score-issue and PV-consume points of attn chunk j's pair loop.
    During pure attention the ScalarE exp stream needs ~1.5x the PE time of
    the scores+PV matmuls, so without weaving the PE starves behind exp in
    the big late chunks (>3.4us idle re-throttles the HAM clock gate to
    1.2 GHz, which then doubles the cost of the final stage_d).  Weaving
    keeps the PE dense and the clock at 2.4 GHz through the tail.

Layout choices:
  - All matmul inputs bf16 (host-cast), f32 PSUM accumulation.
  - x is staged transposed (x^T: [d, s]) so q^T/k^T ([e_local, s]) come
    straight out of the PE and serve as lhsT/rhs of the scores matmul.
  - Scores are computed transposed: S^T[kpos, q], contraction d=64 per
    head, two heads packed in the 128-row PE array via row tiling
    (tile_position (0,0)/(64,0)) writing one 2-bank PSUM pair, so a single
    ScalarE ACTIVATE does exp for both heads (amortizes the 352-cycle
    ACTIVATE overhead).
  - Softmax without max-subtraction (scores are O(10), exp is safe in f32),
    exp output bf16.
  - Causality at tile granularity: kpos-tile i of q-chunk j is skipped when
    fully masked; diagonal tiles shift/shrink to the valid q-range
    (q >= kpos) so scores/exp/PV only touch live columns, and the residual
    mask reduces to a single 128-col triangle (f >= p) on every diagonal
    tile.
  - P @ V via V_aug = [V | 1*64]: lhsT = V_aug [kpos, 128] — columns 64:128
    are all-ones, so PSUM rows 64:128 of the accumulated [128, q] output hold
    the softmax denominator already replicated across 64 partitions (free:
    matmul cost is free-dim cycles, independent of stationary width).  This
    keeps gpsimd partition_broadcast off the critical path.
  - attn^T = out[0:64] * (1/denominator), via fast DVE reciprocal (18-bit;
    the exact one runs at 1/8 rate) on the already-broadcast rows.  Custom
    ops (reciprocal_approx_fast) only get base-partition-0 SBUF inputs —
    they misbehave otherwise on HW.
  - y = attn^T.T @ w_o^T slices, bf16 out (partials are summed on host in
    f32; bf16 partials cost ~4e-3 rel err against the 2e-2 gate).
"""

import numpy as np
import ml_dtypes

B = 2
S = 2048
D = 1024
H = 16
DK = 64
NCORES = 8
EL = 256  # local channels per core (4 heads)
QW = 512  # q-chunk width (free dim of scores matmuls)
NJ = S // QW  # 4 q-chunks

_CACHE = {}


def _build():
    import concourse.bass as bass
    import concourse.mybir as mybir
    import concourse.tile as tile
    from concourse import bacc

    f32 = mybir.dt.float32
    bf16 = mybir.dt.bfloat16
    ts = bass.ts
    Exp = mybir.ActivationFunctionType.Exp

    nc = bacc.Bacc("TRN2", num_devices=NCORES)
    xT_d = nc.dram_tensor("xT", [D, S], bf16, kind="ExternalInput")
    wqT_d = nc.dram_tensor("wqT", [D, EL], bf16, kind="ExternalInput")
    wkT_d = nc.dram_tensor("wkT", [D, EL], bf16, kind="ExternalInput")
    wvT_d = nc.dram_tensor("wvT", [D, EL], bf16, kind="ExternalInput")
    woT_d = nc.dram_tensor("woT", [EL, D], bf16, kind="ExternalInput")
    y_d = nc.dram_tensor("y", [S, D], bf16, kind="ExternalOutput")

    DT = D // 128  # 8 d-tiles
    ST = S // 128  # 16 s-tiles

    with tile.TileContext(nc) as tc:
        with (
            tc.tile_pool(name="big", bufs=1) as big,
            tc.tile_pool(name="work", bufs=3) as work,
            tc.tile_pool(name="psum", bufs=1, space="PSUM") as psum,
        ):
            xT = big.tile([128, DT, S], bf16)  # x^T (d, s)
            wqT = big.tile([128, DT, EL], bf16)
            wkT = big.tile([128, DT, EL], bf16)
            wvT = big.tile([128, DT, EL], bf16)
            woT = big.tile([128, EL // 128, D], bf16)
            qT = big.tile([128, 2, S], bf16)  # (e_local, s)
            kT = big.tile([128, 2, S], bf16)
            vA = big.tile([128, ST, 4, 2 * DK], bf16)  # (s%128, s//128, h, dv|ones)
            aT = big.tile([128, 2, S], bf16)  # attn^T (d_local, s)
            masks = big.tile([128, 1, 128], bf16)  # triangle: f >= p


            # ---- input DMAs, spread over 4 issue queues ordered by first
            # use.  Instruction fetch lands ~5.5us; each jumbo rearranged
            # dma_start costs ~1.5-3us of queue time, so serializing them on
            # one queue delays first data to ~15us.  wq on scalar + x chunk0
            # halves on vector/gpsimd land ~10-11us in parallel. ----
            xT_r = xT_d.ap().rearrange("(ko p) s -> p ko s", p=128)
            nc.scalar.dma_start(
                wqT[:], wqT_d.ap().rearrange("(ko p) e -> p ko e", p=128)
            )
            # warmup scratch memset first on the vector queue (fast) so the
            # PE warmup isn't gated on any DMA.
            warm = work.tile([128, 256], bf16, tag="warm", bufs=1)
            nc.vector.memset(warm[:], 0.0)
            nc.gpsimd.dma_start(xT[:, 0:4, ts(0, QW)], xT_r[:, 0:4, ts(0, QW)])
            nc.scalar.dma_start(xT[:, 4:8, ts(0, QW)], xT_r[:, 4:8, ts(0, QW)])
            nc.sync.dma_start(
                wkT[:], wkT_d.ap().rearrange("(ko p) e -> p ko e", p=128)
            )
            nc.sync.dma_start(
                wvT[:], wvT_d.ap().rearrange("(ko p) e -> p ko e", p=128)
            )
            nc.sync.dma_start(xT[:, :, ts(1, QW)], xT_r[:, :, ts(1, QW)])
            nc.sync.dma_start(
                woT[:], woT_d.ap().rearrange("(ko p) e -> p ko e", p=128)
            )
            for sh in range(2, NJ):
                nc.sync.dma_start(xT[:, :, ts(sh, QW)], xT_r[:, :, ts(sh, QW)])

            # ---- constants (emitted after the DMA issues so the gpsimd
            # queue starts the x loads first): triangle mask + V_aug ones ----
            # mask[p, f] = 1.0 iff f >= p  (diagonal 128-col block; all
            # diagonal kpos-tiles reduce to this after the q-range shift)
            nc.gpsimd.memset(masks[:], 1.0)
            nc.gpsimd.affine_select(
                out=masks[:, 0, :],
                in_=masks[:, 0, :],
                compare_op=mybir.AluOpType.is_ge,
                fill=0.0,
                base=0,
                pattern=[[1, 128]],
                channel_multiplier=-1,
            )
            # ones columns split so chunk 0's PV isn't gated on the full set
            nc.gpsimd.memset(vA[:, 0:4, :, DK : 2 * DK], 1.0)
            nc.gpsimd.memset(vA[:, 4:ST, :, DK : 2 * DK], 1.0)

            # ---- PE warmup: dummy matmuls on zeroed scratch during the DMA
            # wait, so HAM is at 2.4 GHz when real matmuls arrive.  Uses the
            # "sc" slots, which attention only needs much later.
            # input DMAs land ~15-16us (instruction fetch ~5us + issue +
            # ~6us descriptor-fetch latency + transfer); warmup must stay
            # busy until then or the first real matmuls run at half clock.
            for g in range(32):
                wp = psum.tile([128, QW], f32, tag="sc", bufs=2)
                nc.tensor.matmul(wp[:, 0:256], warm[:, 0:128], warm[:])

            def qk_unit(sj, w_sb, outT, et):
                def run():
                    ps = psum.tile([128, QW], f32, tag="mm", bufs=2)
                    for kd in range(DT):
                        nc.tensor.matmul(
                            ps[:],
                            w_sb[:, kd, ts(et, 128)],
                            xT[:, kd, ts(sj, QW)],
                            start=(kd == 0),
                            stop=(kd == DT - 1),
                        )
                    nc.vector.tensor_copy(outT[:, et, ts(sj, QW)], ps[:])

                return run

            def v_unit(st):
                def run():
                    ps = psum.tile([128, EL], f32, tag="mm", bufs=2)
                    for kd in range(DT):
                        nc.tensor.matmul(
                            ps[:],
                            xT[:, kd, ts(st, 128)],
                            wvT[:, kd, :],
                            start=(kd == 0),
                            stop=(kd == DT - 1),
                        )
                    nc.vector.tensor_copy(
                        vA[:, st, :, 0:DK],
                        ps[:].rearrange("p (h e) -> p h e", h=4),
                    )

                return run

            def sd_unit(st, eo, last=False):
                def run():
                    ps = psum.tile([128, QW], f32, tag="mm", bufs=2)
                    for kd in range(EL // 128):
                        nc.tensor.matmul(
                            ps[:],
                            aT[:, kd, ts(st, 128)],
                            woT[:, kd, ts(eo, QW)],
                            start=(kd == 0),
                            stop=(kd == EL // 128 - 1),
                        )
                    yt = work.tile([128, QW], bf16, tag="yout", bufs=4)
                    # final chunk: alternate copy engines so the copy chain
                    # doesn't serialize the last matmul groups (ACT is idle
                    # at the tail, DVE finishes the last divisions)
                    if last and eo == 0:
                        nc.scalar.copy(yt[:], ps[:])
                    else:
                        nc.vector.tensor_copy(yt[:], ps[:])
                    nc.sync.dma_start(
                        y_d.ap().rearrange("(so p) e -> p so e", p=128)[
                            :, st, ts(eo, QW)
                        ],
                        yt[:],
                    )

                return run

            def qk_units(sj):
                return [
                    qk_unit(sj, w_sb, outT, et)
                    for w_sb, outT in ((wqT, qT), (wkT, kT))
                    for et in range(2)
                ]

            def v_units(sj):
                return [v_unit(st) for st in range(4 * sj, 4 * sj + 4)]

            def sd_units(jd, last=False):
                return [
                    sd_unit(st, eo, last)
                    for st in range(4 * jd, 4 * jd + 4)
                    for eo in range(2)
                ]

            def attn_chunk(j, filler, hold=()):
                ilast = 4 * j + 3
                it_total = 4 * (j + 1)  # pair iterations across both hps
                nfill = len(filler)
                it = 0
                last_chunk = j == NJ - 1
                for hp in range(2):  # head pair = e-tile of qT/kT
                    oa0 = psum.tile([128, QW], f32, tag="oa", bufs=2)
                    oa1 = psum.tile([128, QW], f32, tag="oa", bufs=2)
                    # 2-way unrolled over i: both scores pairs are emitted
                    # back-to-back so the row-tiled matmuls can overlap in the
                    # array while the exps of the previous pair run on ScalarE.
                    def geom(i):
                        # diagonal tiles only need q >= kpos: shift the q-range
                        # by off=128r and shrink the matmul/exp/PV width; the
                        # remaining mask is always the 128-col triangle f>=p.
                        if i >= 4 * j:
                            off = 128 * (i - 4 * j)
                            return off, QW - off
                        return 0, QW

                    for ii in range(0, ilast + 1, 2):
                        pair = (ii, ii + 1)
                        scs = {}
                        for i in pair:
                            off, NW = geom(i)
                            sc = psum.tile([128, 2 * QW], f32, tag="sc", bufs=2)
                            nc.tensor.matmul(
                                sc[:, 0:NW],
                                kT[0:64, hp, ts(i, 128)],
                                qT[0:64, hp, bass.ds(j * QW + off, NW)],
                                tile_position=(0, 0),
                            )
                            nc.tensor.matmul(
                                sc[:, QW : QW + NW],
                                kT[64:128, hp, ts(i, 128)],
                                qT[64:128, hp, bass.ds(j * QW + off, NW)],
                                tile_position=(64, 0),
                            )
                            scs[i] = sc
                        # weave PE filler between score issue and PV consume:
                        # exp needs ~1.5x the PE time of this pair's matmuls,
                        # so the PE does independent projection / y work here
                        # instead of stalling on the exp result.  Bresenham
                        # spread so the late (exp-heaviest) iterations still
                        # get filler.
                        npop = ((it + 1) * nfill) // it_total - (it * nfill) // it_total
                        for _ in range(min(npop, len(filler))):
                            filler.pop(0)()
                        it += 1
                        eos = {}
                        for i in pair:
                            off, NW = geom(i)
                            e01 = work.tile(
                                [128, 2 * QW], bf16, tag="exps", bufs=8
                            )
                            sc_v = scs[i][:].rearrange("p (h q) -> p h q", h=2)
                            e_v = e01[:].rearrange("p (h q) -> p h q", h=2)
                            nc.scalar.activation(
                                e_v[:, :, 0:NW], sc_v[:, :, 0:NW], Exp, scale=0.125
                            )
                            if i >= 4 * j:  # diagonal: mask first 128 cols
                                nc.vector.tensor_mul(
                                    e_v[:, :, 0:128],
                                    e_v[:, :, 0:128],
                                    masks[:, 0:1, :].to_broadcast((128, 2, 128)),
                                )
                            eos[i] = e01
                        for i in pair:
                            off, NW = geom(i)
                            nc.tensor.matmul(
                                oa0[:, off : off + NW],
                                vA[:, i, 2 * hp, :],
                                eos[i][:, 0:NW],
                                start=(i == 0),
                                stop=(i == ilast),
                            )
                            nc.tensor.matmul(
                                oa1[:, off : off + NW],
                                vA[:, i, 2 * hp + 1, :],
                                eos[i][:, QW : QW + NW],
                                start=(i == 0),
                                stop=(i == ilast),
                            )
                    fin = last_chunk and hp == 1
                    for hh, oa in ((0, oa0), (1, oa1)):
                        h = 2 * hp + hh
                        dn = work.tile([64, QW], f32, tag="dn", bufs=3)
                        # final head pair: the denominator copies go on the
                        # (by now idle) ScalarE so the DVE chain before the
                        # last stage_d is just recip+mul per head.
                        if fin:
                            nc.scalar.copy(dn[:], oa[DK : 2 * DK, :])
                        else:
                            nc.vector.tensor_copy(dn[:], oa[DK : 2 * DK, :])
                        rc = work.tile([64, QW], f32, tag="rc", bufs=3)
                        nc.vector.reciprocal_approx_fast(out=rc[:], in_=dn[:])
                        nc.vector.tensor_mul(
                            aT[(h % 2) * 64 : (h % 2) * 64 + 64, h // 2, ts(j, QW)],
                            oa[0:DK, :],
                            rc[:],
                        )
                # held-back PE filler: runs while the final normalize chain
                # drains on DVE, keeping the HAM clock at full speed into the
                # last stage_d.
                for u in hold:
                    u()

            # ---- the pipeline ----
            for u in qk_units(0):
                u()
            for u in v_units(0):
                u()
            attn_chunk(0, qk_units(1))
            for j in range(1, NJ):
                filler = v_units(j) + sd_units(j - 1)
                hold = ()
                if j + 1 < NJ:
                    filler += qk_units(j + 1)
                else:
                    filler, hold = filler[:-3], filler[-3:]
                attn_chunk(j, filler, hold)
            for u in sd_units(NJ - 1, last=True):
                u()

    nc.compile()
    return nc


def _get_nc():
    if "nc" not in _CACHE:
        _CACHE["nc"] = _build()
    return _CACHE["nc"]


def kernel(x, w_q, w_k, w_v, w_o, _trace=False, _trace_cores=None):
    from concourse.bass_utils import run_bass_kernel_spmd

    nc = _get_nc()
    bf = ml_dtypes.bfloat16
    in_maps = []
    for c in range(NCORES):
        b = c // 4
        g = c % 4
        ch = slice(g * EL, (g + 1) * EL)
        in_maps.append(
            {
                "xT": np.ascontiguousarray(x[b].T).astype(bf),
                "wqT": np.ascontiguousarray(w_q[ch, :].T).astype(bf),
                "wkT": np.ascontiguousarray(w_k[ch, :].T).astype(bf),
                "wvT": np.ascontiguousarray(w_v[ch, :].T).astype(bf),
                "woT": np.ascontiguousarray(w_o[:, ch].T).astype(bf),
            }
        )
    res = run_bass_kernel_spmd(
        nc,
        in_maps,
        core_ids=list(range(NCORES)),
        trace=_trace,
        trace_cores=_trace_cores,
    )
    _CACHE["last_results"] = res
    y = np.zeros((B, S, D), np.float32)
    for c in range(NCORES):
        y[c // 4] += res.results[c]["y"].astype(np.float32)
    return y

